# revision 7
# baseline (speedup 1.0000x reference)
"""Trainium2 Bass kernel for nn_DensityLoss (row-sharded SPMD x8).

Math
----
reference(centers, features, labels) depends only on centers [C=4096, D=256]
(features unused; labels only via N=len(labels)=262144, a constant):

    sq_i  = ||c_i||^2;  m = sum_i c_i;  S = sum sq;  Sigma = X'X
    n_i   = C*sq_i + S - 2*c_i.m        (center_dist_i = n_i/(C-1); diag==0)
    sum n   = 2*C*S - 2*m.m
    sum n^2 = C^2 q + 3C S^2 - 4C (w.m) - 4S (m.m) + 4 m'Sigma m
        q = sum sq^2, w = sum sq_i c_i
    var = (sum n^2/(C-1)^2 - (sum n/(C-1))^2/C)/(C-1)
    result = (sum n/(C-1))/C/var/N

Sharding: centers row-sharded 512 rows/core across 8 cores.  All needed
globals are linear in per-core partials, so each core emits its partial
Gram of A_k=[X_k | 1 | sq/256] (bf16 products, f32 psum) plus exact-f32
per-partition S'/q' partials, and the host sums 8 small outputs in float64
and finishes the scalar.  This cuts chip HBM traffic 8x vs the previous
replicated kernel (whose 4MiB/core DMA saturated chip HBM at ~25us/round).

Per-core device work (one round):
  - DMA in 512KB as 4 tile-chunks on two HWDGE rings (sync: 0,2; ACT: 1,3)
  - DVE per tile: sq' accum (f32 STT), X cast f32->bf16, sq' col cast
  - PE: A-pass psA = G[0:128, 0:258] (4 mm), B-pass psB = G[128:256,
    128:258] (4 mm, Gram symmetry halves the second block)
  - out: gA [128,258], gB [128,130] (psum->SBUF copies on DVE/ACT), sv
    [128,2] f32 (S', 2^20 q' per-partition partials; q' prescale exact)

"serial" rounds chain round r+1's first DMA on round r's last output DMA
for slope timing (end-to-end latency per round, immune to launch overhead).
"""

import numpy as np

C, D = 4096, 256
N_LABELS = 262144
P = 128
ROWS = C // 8          # 512 rows per core
NT = ROWS // P         # 4 row tiles
W = D + 2              # 258: [X | ones | sq']
WP = 264               # padded SBUF row stride (32B-aligned)
WB = 130               # psB width: G[128:256, 128:258]
N_CORES = 8

_CACHE = {}


def _build_nc(rounds=1, mode="serial"):
    import concourse.bass as bass
    from concourse import mybir

    f32 = mybir.dt.float32
    bf16 = mybir.dt.bfloat16
    Alu = mybir.AluOpType

    nc = bass.Bass()
    x_ext = nc.declare_dram_parameter("centers", [ROWS, D], f32,
                                      isOutput=False)
    ga_ext = nc.declare_dram_parameter("gA", [P, W], f32, isOutput=True)
    gb_ext = nc.declare_dram_parameter("gB", [P, WB], f32, isOutput=True)
    sv_ext = nc.declare_dram_parameter("sv", [P, 2], f32, isOutput=True)

    xv = x_ext[:, :].rearrange("(p t) d -> p t d", p=P)   # [128, 4, 256]

    from contextlib import ExitStack

    with ExitStack() as ctx:
        en = ctx.enter_context
        xh = en(nc.sbuf_tensor([P, NT, WP], f32))
        xhb = en(nc.sbuf_tensor([P, NT, WP], bf16))
        scrv = en(nc.sbuf_tensor([P, D], f32))     # DVE STT main-out sink
        sq2 = en(nc.sbuf_tensor([P, NT], f32))
        gAs = en(nc.sbuf_tensor([P, W], f32))
        gBs = en(nc.sbuf_tensor([P, WB], f32))
        e = en(nc.sbuf_tensor([P, 2], f32))
        psA = en(nc.psum_tensor([P, W], f32))
        psB = en(nc.psum_tensor([P, WB], f32))
        s_dma = [en(nc.semaphore(f"s_dma{i}")) for i in range(NT)]
        s_pre = en(nc.semaphore("s_pre"))
        s_tile = en(nc.semaphore("s_tile"))
        s_pa = en(nc.semaphore("s_pa"))
        s_pb = en(nc.semaphore("s_pb"))
        s_ga = en(nc.semaphore("s_ga"))
        s_gb = en(nc.semaphore("s_gb"))
        s_sv = en(nc.semaphore("s_sv"))
        s_out = en(nc.semaphore("s_out"))
        block = en(nc.Block())

        sqv = xh[:, :, D + 1]                      # [128, 4] sq' column

        @block.sync
        def _(sync):
            for r in range(rounds):
                for t in (0, 2):
                    if t == 0 and r > 0:
                        # serial chain: full drain of round r-1
                        sync.wait_ge(s_out, 48 * r)
                    sync.dma_start(
                        out=xh[:, t, 0:D], in_=xv[:, t, :],
                    ).then_inc(s_dma[t], 16)
                sync.wait_ge(s_sv, r + 1)
                sync.dma_start(out=sv_ext[:, :],
                               in_=e[:, :]).then_inc(s_out, 16)
                sync.wait_ge(s_ga, r + 1)
                sync.dma_start(out=ga_ext[:, :],
                               in_=gAs[:, :]).then_inc(s_out, 16)
                sync.wait_ge(s_gb, r + 1)
                sync.dma_start(out=gb_ext[:, :],
                               in_=gBs[:, :]).then_inc(s_out, 16)
            sync.wait_ge(s_out, 48 * rounds)

        @block.vector
        def _(vector):
            # preamble: bf16 ones column (disjoint from DMA'd cols 0:256)
            nc.vector.memset(xhb[:, :, D:D + 1], 1.0).then_inc(s_pre, 1)
            STT = nc.vector.scalar_tensor_tensor
            for r in range(rounds):
                for t in range(NT):
                    vector.wait_ge(s_dma[t], 16 * (r + 1))
                    # sq' = sum((x/256)*x) f32 accum into xh col 257
                    STT(scrv[:, :], xh[:, t, 0:D], 1.0 / 256.0,
                        xh[:, t, 0:D], op0=Alu.mult, op1=Alu.mult,
                        accum_out=xh[:, t, D + 1:D + 2])
                    # X cast for PE
                    nc.vector.tensor_copy(out=xhb[:, t, 0:D],
                                          in_=xh[:, t, 0:D])
                    # drain flushes the STT accumulator write (RAW hazard:
                    # without it the sq' cast can read a stale col 257)
                    vector.drain()
                    nc.vector.tensor_copy(
                        out=xhb[:, t, D + 1:D + 2],
                        in_=xh[:, t, D + 1:D + 2],
                    ).then_inc(s_tile, 1)
                # S' and 2^20*q' per-partition partials (f32, exact scale)
                nc.vector.tensor_reduce(e[:, 0:1], sqv,
                                        axis=mybir.AxisListType.X, op=Alu.add)
                STT(sq2[:, :], sqv, 2.0 ** 20, sqv, op0=Alu.mult,
                    op1=Alu.mult)
                vector.drain()
                nc.vector.tensor_reduce(e[:, 1:2], sq2[:, :],
                                        axis=mybir.AxisListType.X,
                                        op=Alu.add).then_inc(s_sv, 1)
                vector.wait_ge(s_pa, r + 1)
                nc.vector.tensor_copy(gAs[:, :], psA[:, :]).then_inc(s_ga, 1)

        @block.scalar
        def _(scalar):
            for r in range(rounds):
                # odd chunks on the second HWDGE ring
                for t in (1, 3):
                    if t == 1 and r > 0:
                        scalar.wait_ge(s_out, 48 * r)
                    nc.scalar.dma_start(
                        out=xh[:, t, 0:D], in_=xv[:, t, :],
                    ).then_inc(s_dma[t], 16)
                scalar.wait_ge(s_pb, r + 1)
                nc.scalar.copy(gBs[:, :], psB[:, :]).then_inc(s_gb, 1)

        @block.tensor
        def _(tensor):
            tensor.wait_ge(s_pre, 1)
            for r in range(rounds):
                for t in range(NT):
                    tensor.wait_ge(s_tile, NT * r + t + 1)
                    mm = nc.tensor.matmul(psA[:, :], xhb[:, t, 0:P],
                                          xhb[:, t, 0:W], start=(t == 0),
                                          stop=(t == NT - 1))
                    if t == NT - 1:
                        mm.then_inc(s_pa, 1)
                for t in range(NT):
                    mm = nc.tensor.matmul(psB[:, :], xhb[:, t, P:D],
                                          xhb[:, t, P:P + WB],
                                          start=(t == 0),
                                          stop=(t == NT - 1))
                    if t == NT - 1:
                        mm.then_inc(s_pb, 1)

    return nc


def _get_nc(rounds=1, mode="serial"):
    key = ("nc", rounds, mode)
    if key not in _CACHE:
        _CACHE[key] = _build_nc(rounds, mode)
    return _CACHE[key]


def _finish(results):
    """Sum 8 per-core partials in float64 and finish the scalar on host."""
    GA = np.zeros((P, W), np.float64)
    GB = np.zeros((P, WB), np.float64)
    S_ = 0.0
    q_ = 0.0
    for res in results:
        GA += np.asarray(res["gA"], np.float64)
        GB += np.asarray(res["gB"], np.float64)
        sv = np.asarray(res["sv"], np.float64)
        S_ += sv[:, 0].sum()
        q_ += sv[:, 1].sum()
    Cf = float(C)
    S = 256.0 * S_
    q = q_ / 16.0                      # 2^20 * (sq/256)^2 = 16 sq^2
    m = np.concatenate([GA[:, 256], GB[:, 128]])
    wp = np.concatenate([GA[:, 257], GB[:, 129]])
    Sig = np.zeros((D, D), np.float64)
    Sig[0:P, :] = GA[:, 0:D]
    Sig[P:D, P:D] = GB[:, 0:P]
    Sig[P:D, 0:P] = GA[:, P:D].T
    mm = m @ m
    mSm = m @ (Sig @ m)
    wm = 256.0 * (wp @ m)
    sum_n = 2.0 * Cf * S - 2.0 * mm
    sum_n2 = (Cf * Cf * q + 3.0 * Cf * S * S - 4.0 * Cf * wm
              - 4.0 * S * mm + 4.0 * mSm)
    cd_sum = sum_n / (Cf - 1.0)
    cd_sq = sum_n2 / (Cf - 1.0) ** 2
    var = (cd_sq - cd_sum * cd_sum / Cf) / (Cf - 1.0)
    res = cd_sum / Cf / var / float(N_LABELS)
    return np.float32(res).reshape(())


def run(centers: np.ndarray, trace: bool = False):
    """Run the SPMD kernel on cores 0-7; returns (scalar ndarray, results)."""
    from concourse.bass_utils import run_bass_kernel_spmd

    nc = _get_nc()
    x = np.ascontiguousarray(np.asarray(centers, dtype=np.float32))
    in_maps = [{"centers": x[ROWS * k:ROWS * (k + 1)]}
               for k in range(N_CORES)]
    r = run_bass_kernel_spmd(nc, in_maps, core_ids=list(range(N_CORES)),
                             trace=trace)
    return _finish(r.results), r


def kernel(centers, features=None, labels=None, **_):
    out, _r = run(centers)
    return out


# revision 16
# speedup vs baseline: 2.6552x; 2.6552x over previous
"""Trainium2 Bass kernel for nn_DensityLoss (column-sharded SPMD x8).

Math
----
reference(centers, features, labels) depends only on centers [C=4096, D=256]
(features unused; labels only via N=len(labels)=262144, a constant):

    sq_i = ||c_i||^2;  m = sum_i c_i;  S = sum sq
    n_i  = C*sq_i + S - 2*c_i.m        (center_dist_i = n_i/(C-1); diag==0)
    sum n   = 2*C*S - 2*m.m
    sum n^2 = C^2 q + 3C S^2 - 4C (w.m) - 4S (m.m) + 4 m'Sigma m
        q = sum sq_i^2,  w.m = sum_i sq_i p_i,  m'Sigma m = sum_i p_i^2,
        p_i = c_i.m
    var = (sum n^2/(C-1)^2 - (sum n/(C-1))^2/C)/(C-1)
    result = (sum n/(C-1))/C/var/N

Sharding: centers COLUMN-sharded, 32 columns per core.  On this stack DMA
writes to DRAM run at ~2.3 GB/s (reads are ~140 GB/s/ring), so per-core
outputs must be tiny; column sharding makes every cross-core coupling a
length-C vector of zero-mean residuals, shippable in low precision:

  core c (local slice Xc [4096, 32], mc = its 32 entries of m -- exact):
    yv_i = c_i^c . mc            (partial row-dot; p_i = sum_c yv_i)
    rv_i = ||c_i^c||^2 - 32      (partial sq residual; sq_i = 256 + sum_c rv)
  outputs: yv, rv cast to VEC_DT [128, 64] + exact f32 scalars
  [S_c, q_cc = sum rv^2, Y2_c = sum yv^2, mc] for the cancellation-critical
  terms.  Host assembles everything in float64; the vector precision only
  touches cross-core cross terms (error ~1e-3 final, gate is 2e-2).

Device per round: DMA in 512KB (2 rings) -> DVE: squares/reduces, row-dot
via one broadcast-multiply + reduce -> PE: three tiny fp32 matmuls for
partition reductions (m, m-broadcast, scalar gather) -> 3 small out-DMAs.
"serial" rounds chain round r+1's first DMA on round r's last output DMA
for slope timing (end-to-end latency per round, immune to launch overhead).
"""

import numpy as np

C, D = 4096, 256
N_LABELS = 262144
P = 128
DC = D // 8            # 32 columns per core
NT = C // P            # 32 row tiles (rows per partition)
HT = NT // 2           # DMA chunk: 16 tiles
N_CORES = 8

VEC_DT = "bfloat16"    # dtype of the y/r residual vectors ("float8e4" opt)

_CACHE = {}


def _build_nc(rounds=1, mode="serial"):
    import concourse.bass as bass
    from concourse import mybir

    f32 = mybir.dt.float32
    vdt = getattr(mybir.dt, VEC_DT)
    Alu = mybir.AluOpType
    AX = mybir.AxisListType

    nc = bass.Bass()
    x_ext = nc.declare_dram_parameter("centers", [C, DC], f32,
                                      isOutput=False)
    oy_ext = nc.declare_dram_parameter("oy", [P, NT], vdt, isOutput=True)
    or_ext = nc.declare_dram_parameter("orr", [P, NT], vdt, isOutput=True)
    of_ext = nc.declare_dram_parameter("of", [1, 40], f32, isOutput=True)

    xv = x_ext[:, :].rearrange("(p t) d -> p t d", p=P)   # [128, 32, 32]

    from contextlib import ExitStack

    with ExitStack() as ctx:
        en = ctx.enter_context
        xh = en(nc.sbuf_tensor([P, NT, DC], f32))
        xsq = en(nc.sbuf_tensor([P, NT * DC], f32))    # also reused for prod
        mbig = en(nc.sbuf_tensor([P, NT * DC], f32))   # m tiled x32
        sqv = en(nc.sbuf_tensor([P, NT], f32))
        rv = en(nc.sbuf_tensor([P, NT], f32))
        yv = en(nc.sbuf_tensor([P, NT], f32))
        xs = en(nc.sbuf_tensor([P, DC], f32))
        cols = en(nc.sbuf_tensor([P, 3], f32))
        ones_col = en(nc.sbuf_tensor([P, 1], f32))
        ones_row = en(nc.sbuf_tensor([1, P], f32))
        o16 = en(nc.sbuf_tensor([P, 2 * NT], vdt))
        msb = en(nc.sbuf_tensor([1, DC], f32))
        ob = en(nc.sbuf_tensor([1, 40], f32))
        psM = en(nc.psum_tensor([1, DC], f32))
        psMb = en(nc.psum_tensor([P, DC], f32))
        psF = en(nc.psum_tensor([1, 3], f32))
        s_dma = [en(nc.semaphore(f"s_dma{i}")) for i in range(2)]
        s_pre = en(nc.semaphore("s_pre"))
        s_r = en(nc.semaphore("s_r"))
        s_xs = en(nc.semaphore("s_xs"))
        s_m = en(nc.semaphore("s_m"))
        s_msb = en(nc.semaphore("s_msb"))
        s_mb = en(nc.semaphore("s_mb"))
        s_y = en(nc.semaphore("s_y"))
        s_cols = en(nc.semaphore("s_cols"))
        s_f = en(nc.semaphore("s_f"))
        s_ob = en(nc.semaphore("s_ob"))
        s_out = en(nc.semaphore("s_out"))
        block = en(nc.Block())

        @block.sync
        def _(sync):
            for r in range(rounds):
                if r > 0:
                    sync.wait_ge(s_out, 48 * r)
                sync.dma_start(
                    out=xh[:, 0:HT, :], in_=xv[:, 0:HT, :],
                ).then_inc(s_dma[0], 16)
                # r-residual out fires first (ready before the y path)
                sync.wait_ge(s_r, r + 1)
                sync.dma_start(out=or_ext[:, :],
                               in_=o16[:, NT:2 * NT]).then_inc(s_out, 16)
                sync.wait_ge(s_y, r + 1)
                sync.dma_start(out=oy_ext[:, :],
                               in_=o16[:, 0:NT]).then_inc(s_out, 16)
                sync.wait_ge(s_ob, r + 1)
                sync.dma_start(out=of_ext[:, :],
                               in_=ob[:, :]).then_inc(s_out, 16)
            sync.wait_ge(s_out, 48 * rounds)

        @block.scalar
        def _(scalar):
            for r in range(rounds):
                if r > 0:
                    scalar.wait_ge(s_out, 48 * r)
                nc.scalar.dma_start(
                    out=xh[:, HT:NT, :], in_=xv[:, HT:NT, :],
                ).then_inc(s_dma[1], 16)

        @block.vector
        def _(vector):
            nc.vector.memset(ones_col[:, :], 1.0)
            nc.vector.memset(ones_row[:, :], 1.0).then_inc(s_pre, 1)
            TT = nc.vector.tensor_tensor
            TS = nc.vector.tensor_scalar
            STT = nc.vector.scalar_tensor_tensor
            for r in range(rounds):
                # --- sq path (chunked overlap with DMA) ---
                for h in range(2):
                    lo, hi = h * HT, (h + 1) * HT
                    vector.wait_ge(s_dma[h], 16 * (r + 1))
                    TT(xsq[:, lo * DC:hi * DC].rearrange(
                           "p (t d) -> p t d", t=HT),
                       xh[:, lo:hi, :],
                       xh[:, lo:hi, :], op=Alu.mult)
                    vector.drain()
                    nc.vector.tensor_reduce(
                        sqv[:, lo:hi],
                        xsq[:, lo * DC:hi * DC].rearrange(
                            "p (t d) -> p t d", t=HT),
                        axis=AX.X, op=Alu.add)
                vector.drain()
                TS(rv[:, :], sqv[:, :], -32.0, None, op0=Alu.add)
                vector.drain()
                nc.vector.tensor_copy(out=o16[:, NT:2 * NT],
                                      in_=rv[:, :]).then_inc(s_r, 1)
                # exact f32 scalar partials: S_c, q_cc
                nc.vector.tensor_reduce(cols[:, 0:1], sqv[:, :],
                                        axis=AX.X, op=Alu.add)
                STT(xsq[:, 0:NT], rv[:, :], 1.0, rv[:, :],
                    op0=Alu.mult, op1=Alu.mult)
                vector.drain()
                nc.vector.tensor_reduce(cols[:, 1:2], xsq[:, 0:NT],
                                        axis=AX.X, op=Alu.add)
                # --- m path: xs[p, d] = sum_t x[p, t, d] ---
                nc.vector.tensor_reduce(
                    xs[:, :],
                    xh[:, :, :].rearrange("p t d -> p d t"),
                    axis=AX.X, op=Alu.add).then_inc(s_xs, 1)
                vector.wait_ge(s_m, r + 1)
                nc.vector.tensor_copy(out=msb[:, :],
                                      in_=psM[0:1, :]).then_inc(s_msb, 1)
                # mbig = m tiled x32 (log2 doubling)
                vector.wait_ge(s_mb, r + 1)
                nc.vector.tensor_copy(out=mbig[:, 0:DC], in_=psMb[:, :])
                w = DC
                while w < NT * DC:
                    vector.drain()
                    nc.vector.tensor_copy(out=mbig[:, w:2 * w],
                                          in_=mbig[:, 0:w])
                    w *= 2
                vector.drain()
                # --- y path ---
                TT(xsq[:, :].rearrange("p (t d) -> p t d", t=NT),
                   xh[:, :, :],
                   mbig[:, :].rearrange("p (t d) -> p t d", t=NT),
                   op=Alu.mult)
                vector.drain()
                nc.vector.tensor_reduce(
                    yv[:, :],
                    xsq[:, :].rearrange("p (t d) -> p t d", t=NT),
                    axis=AX.X, op=Alu.add)
                vector.drain()
                nc.vector.tensor_copy(out=o16[:, 0:NT],
                                      in_=yv[:, :]).then_inc(s_y, 1)
                STT(xsq[:, 0:NT], yv[:, :], 1.0, yv[:, :],
                    op0=Alu.mult, op1=Alu.mult)
                vector.drain()
                nc.vector.tensor_reduce(cols[:, 2:3], xsq[:, 0:NT],
                                        axis=AX.X,
                                        op=Alu.add).then_inc(s_cols, 1)
                vector.wait_ge(s_f, r + 1)
                nc.vector.tensor_copy(out=ob[:, 0:3], in_=psF[0:1, :])
                nc.vector.tensor_copy(out=ob[:, 8:8 + DC],
                                      in_=msb[:, :]).then_inc(s_ob, 1)

        @block.tensor
        def _(tensor):
            tensor.wait_ge(s_pre, 1)
            for r in range(rounds):
                tensor.wait_ge(s_xs, r + 1)
                nc.tensor.matmul(psM[:, :], ones_col[:, :], xs[:, :],
                                 start=True, stop=True).then_inc(s_m, 1)
                tensor.wait_ge(s_msb, r + 1)
                nc.tensor.matmul(psMb[:, :], ones_row[:, :], msb[:, :],
                                 start=True, stop=True).then_inc(s_mb, 1)
                tensor.wait_ge(s_cols, r + 1)
                nc.tensor.matmul(psF[:, :], ones_col[:, :], cols[:, :],
                                 start=True, stop=True).then_inc(s_f, 1)

    return nc


def _get_nc(rounds=1, mode="serial"):
    key = ("nc", rounds, mode)
    if key not in _CACHE:
        _CACHE[key] = _build_nc(rounds, mode)
    return _CACHE[key]


def _finish(results):
    """Assemble 8 per-core partials in float64; finish the scalar on host."""
    Cf = float(C)
    m = np.zeros(D, np.float64)
    S = 0.0          # sum_i sq_i (exact)
    q_diag = 0.0     # sum_c sum_i (r_i^c)^2 (exact)
    Y2 = 0.0         # sum_c sum_i (y_i^c)^2 (exact)
    R = np.zeros(C, np.float64)    # sum_c r^c (vector dtype precision)
    Pv = np.zeros(C, np.float64)   # sum_c y^c
    R2d = 0.0        # sum_c sum_i (r8_i^c)^2 (vector-dtype diag, to subtract)
    Y2d = 0.0
    for c, res in enumerate(results):
        of = np.asarray(res["of"], np.float64).reshape(-1)
        S += of[0]
        q_diag += of[1]
        Y2 += of[2]
        m[DC * c:DC * (c + 1)] = of[8:8 + DC]
        yc = np.asarray(res["oy"], np.float64).reshape(C)
        rc = np.asarray(res["orr"], np.float64).reshape(C)
        Pv += yc
        R += rc
        Y2d += float(yc @ yc)
        R2d += float(rc @ rc)
    mm = float(m @ m)
    # q = sum (256 + R_i)^2 with exact diagonal substitution
    SR = S - 32.0 * 8 * C                     # sum_i R_i (exact via S_c)
    R2 = float(R @ R) - R2d + q_diag          # sum R_i^2, exact diagonal
    q = C * 256.0 * 256.0 + 2.0 * 256.0 * SR + R2
    # m'Sigma m = sum p_i^2 with exact diagonal
    mSm = float(Pv @ Pv) - Y2d + Y2
    # w.m = sum sq_i p_i = 256*sum(p) + sum R_i p_i; sum(p) = m.m exactly
    wm = 256.0 * mm + float(R @ Pv)
    sum_n = 2.0 * Cf * S - 2.0 * mm
    sum_n2 = (Cf * Cf * q + 3.0 * Cf * S * S - 4.0 * Cf * wm
              - 4.0 * S * mm + 4.0 * mSm)
    cd_sum = sum_n / (Cf - 1.0)
    cd_sq = sum_n2 / (Cf - 1.0) ** 2
    var = (cd_sq - cd_sum * cd_sum / Cf) / (Cf - 1.0)
    res = cd_sum / Cf / var / float(N_LABELS)
    return np.float32(res).reshape(())


def run(centers: np.ndarray, trace: bool = False):
    """Run the SPMD kernel on cores 0-7; returns (scalar ndarray, results)."""
    from concourse.bass_utils import run_bass_kernel_spmd

    nc = _get_nc()
    x = np.asarray(centers, dtype=np.float32)
    in_maps = [
        {"centers": np.ascontiguousarray(x[:, DC * k:DC * (k + 1)])}
        for k in range(N_CORES)
    ]
    r = run_bass_kernel_spmd(nc, in_maps, core_ids=list(range(N_CORES)),
                             trace=trace)
    return _finish(r.results), r


def kernel(centers, features=None, labels=None, **_):
    out, _r = run(centers)
    return out
